# revision 8
# baseline (speedup 1.0000x reference)
"""Trainium2 Bass kernel for the OOD-detection kNN head (Poincare-ball 5-NN).

Math: for queries z (4096,256) and leaves y (50000,256) inside the unit ball,
the reference ranks leaves per query by the Poincare distance
  d = arccosh(1 + 2*|x-y|^2 / ((1-|x|^2)(1-|y|^2)))
Using the hyperboloid identity,
  cosh d = [(1+x2)(1+y2) - 4 z.y] / [(1-x2)(1-y2)]
so per query the ranking is equivalent to minimizing
  G'[b,l] = Y0[l] - z'_b . Y4_l
with z' = z/(1+x2) (host precompute), Y4 = 4*y/(1-y2), Y0 = (1+y2)/(1-y2).
The device maximizes H = z'.Y4 - Y0 via a K=256 fp32 matmul plus a K=1
bias matmul, reduces H by max over groups of G=32 consecutive leaves, and
returns the top-8 group values + indices per query per core. The host merges
the 64 candidate groups per query, recomputes exact fp32 distances for the
member leaves of the best groups, and emits top-5 ids / min score / OOD flag.

Sharding: leaves split across 8 cores (padded 50000 -> 8*6656), queries
replicated. Output per core is tiny (4096 x 8 x 2), merged on host.
"""

import sys

for _p in ("/opt/trn_rl_repo", "/root/.axon_site/_ro/trn_rl_repo"):
    if _p not in sys.path:
        sys.path.append(_p)

import numpy as np

import concourse.bass as bass
import concourse.bacc as bacc
import concourse.mybir as mybir
from concourse.tile import TileContext

B = 4096          # queries
D = 256           # embedding dim
L = 50000         # real leaves
NCORES = 8
LSH = 6656        # leaves per core (padded)
LPAD = NCORES * LSH
G = 32            # leaves per candidate group
NGROUP = LSH // G  # 208 groups per core
P = 128
NB = B // P       # 32 query blocks
CHUNK = 512       # leaf columns per matmul
NCHUNK = LSH // CHUNK  # 13
GPC = CHUNK // G  # 16 groups per chunk
TOPG = 8          # merged groups whose members get exact recompute
EPS = 1e-7
PAD_NEG = -1e30

MMDT = mybir.dt.float32r  # fp32 "replicated" matmul: full precision, 1 cyc/row
F32 = mybir.dt.float32
U32 = mybir.dt.uint32


def build_nc():
    nc = bacc.Bacc()
    zs_d = nc.declare_dram_parameter("zs", [2, P, B], MMDT, isOutput=False)
    lf_d = nc.declare_dram_parameter("lf", [2, P, LSH], MMDT, isOutput=False)
    ny0_d = nc.declare_dram_parameter("ny0", [1, LSH], MMDT, isOutput=False)
    ones_d = nc.declare_dram_parameter("ones", [1, P], MMDT, isOutput=False)
    outv_d = nc.declare_dram_parameter("outv", [NB, P, 8], F32, isOutput=True)
    outi_d = nc.declare_dram_parameter("outi", [NB, P, 8], U32, isOutput=True)

    with TileContext(nc) as tc:
        with (
            tc.tile_pool(name="const", bufs=1) as cpool,
            tc.tile_pool(name="zp", bufs=1) as zpool,
            tc.tile_pool(name="lp", bufs=1) as lpool,
            tc.tile_pool(name="red", bufs=3) as rpool,
            tc.tile_pool(name="o8", bufs=4) as opool,
            tc.tile_pool(name="ps", bufs=8, space="PSUM") as ppool,
        ):
            ones_t = cpool.tile([1, P], MMDT)
            nc.sync.dma_start(out=ones_t, in_=ones_d[:, :])
            ny0_t = cpool.tile([1, LSH], MMDT)
            nc.sync.dma_start(out=ny0_t, in_=ny0_d[:, :])

            z_t = zpool.tile([P, 2, B], MMDT)
            nc.sync.dma_start(
                out=z_t, in_=zs_d.rearrange("h p b -> p h b")
            )

            leaf_ts = []
            for cc in range(NCHUNK):
                lt = lpool.tile([P, 2, CHUNK], MMDT, name=f"leaf{cc}")
                nc.sync.dma_start(
                    out=lt,
                    in_=lf_d[:, :, cc * CHUNK:(cc + 1) * CHUNK].rearrange(
                        "h p c -> p h c"
                    ),
                )
                leaf_ts.append(lt)

            for bb in range(NB):
                zsl = (slice(None), slice(bb * P, (bb + 1) * P))
                acc = rpool.tile([P, NGROUP], F32, tag="acc")
                for cc in range(NCHUNK):
                    ps = ppool.tile([P, CHUNK], F32, tag="ps")
                    nc.tensor.matmul(
                        ps, z_t[:, 0, zsl[1]], leaf_ts[cc][:, 0, :],
                        start=True, stop=False,
                    )
                    nc.tensor.matmul(
                        ps, z_t[:, 1, zsl[1]], leaf_ts[cc][:, 1, :],
                        start=False, stop=False,
                    )
                    nc.tensor.matmul(
                        ps, ones_t, ny0_t[:, cc * CHUNK:(cc + 1) * CHUNK],
                        start=False, stop=True,
                    )
                    nc.vector.tensor_reduce(
                        out=acc[:, cc * GPC:(cc + 1) * GPC],
                        in_=ps.rearrange("p (j g) -> p j g", g=G),
                        axis=mybir.AxisListType.X,
                        op=mybir.AluOpType.max,
                    )
                vals_t = opool.tile([P, 8], F32, tag="vals")
                idx_t = opool.tile([P, 8], U32, tag="idx")
                nc.vector.max(out=vals_t, in_=acc)
                nc.vector.max_index(out=idx_t, in_max=vals_t, in_values=acc)
                nc.sync.dma_start(out=outv_d[bb], in_=vals_t)
                nc.sync.dma_start(out=outi_d[bb], in_=idx_t)
    nc.finalize()
    return nc


_NC = None


def _get_nc():
    global _NC
    if _NC is None:
        _NC = build_nc()
    return _NC


def preprocess(z_hyp, leaf_emb):
    z = np.ascontiguousarray(z_hyp, dtype=np.float32)
    y = np.ascontiguousarray(leaf_emb, dtype=np.float32)
    x2 = np.einsum("bd,bd->b", z, z, dtype=np.float32)
    zp = (z / (1.0 + x2)[:, None]).astype(np.float32)
    zs = np.ascontiguousarray(zp.T.reshape(2, P, B))

    y2 = np.einsum("ld,ld->l", y, y, dtype=np.float32)
    Y4 = (4.0 * y / (1.0 - y2)[:, None]).astype(np.float32)
    Y0 = ((1.0 + y2) / (1.0 - y2)).astype(np.float32)
    Y4p = np.zeros((LPAD, D), np.float32)
    Y4p[:L] = Y4
    nY0p = np.full((LPAD,), PAD_NEG, np.float32)
    nY0p[:L] = -Y0

    in_maps = []
    for c in range(NCORES):
        sl = slice(c * LSH, (c + 1) * LSH)
        lf = np.ascontiguousarray(Y4p[sl].T.reshape(2, P, LSH))
        ny0 = np.ascontiguousarray(nY0p[sl].reshape(1, LSH))
        in_maps.append(
            {"zs": zs, "lf": lf, "ny0": ny0, "ones": np.ones((1, P), np.float32)}
        )
    return in_maps, (z, y, x2, y2)


def run_device(in_maps, trace=False, **kw):
    from concourse.bass_utils import run_bass_kernel_spmd

    return run_bass_kernel_spmd(
        _get_nc(), in_maps, list(range(NCORES)), trace=trace, **kw
    )


def postprocess(results, z, y, x2, y2, leaf_node_ids, threshold):
    ids = np.asarray(leaf_node_ids)
    thr = np.float32(threshold)

    vals = np.stack(
        [results[c]["outv"].reshape(B, 8) for c in range(NCORES)], axis=1
    ).reshape(B, NCORES * 8)
    gidx = np.stack(
        [
            results[c]["outi"].reshape(B, 8).astype(np.int64) + c * NGROUP
            for c in range(NCORES)
        ],
        axis=1,
    ).reshape(B, NCORES * 8)

    order = np.argsort(-vals, axis=1, kind="stable")[:, :TOPG]
    sel = np.take_along_axis(gidx, order, axis=1)            # (B, TOPG)

    leaf_idx = (sel[:, :, None] * G + np.arange(G)[None, None, :]).reshape(
        B, TOPG * G
    )
    valid = leaf_idx < L
    li = np.where(valid, leaf_idx, 0)

    scores = np.empty(B, np.float32)
    topk_pos = np.empty((B, 5), np.int64)
    CH = 512
    for b0 in range(0, B, CH):
        b1 = min(b0 + CH, B)
        lic = li[b0:b1]
        yl = y[lic]                                           # (ch, TG, 256)
        xy = np.einsum("bd,bkd->bk", z[b0:b1], yl, dtype=np.float32)
        sq = np.maximum(
            x2[b0:b1][:, None] + y2[lic] - 2.0 * xy, 0.0
        ).astype(np.float32)
        den = np.maximum(
            (1.0 - x2[b0:b1][:, None]) * (1.0 - y2[lic]), EPS
        ).astype(np.float32)
        arg = np.maximum(1.0 + 2.0 * sq / den, 1.0 + EPS).astype(np.float32)
        d = np.arccosh(arg).astype(np.float32)
        d = np.where(valid[b0:b1], d, np.float32(np.inf))
        ord2 = np.lexsort((lic, d), axis=1)[:, :5]
        topk_pos[b0:b1] = np.take_along_axis(lic, ord2, axis=1)
        scores[b0:b1] = np.take_along_axis(d, ord2, axis=1)[:, 0]

    topk_ids = ids[topk_pos].astype(np.int32)
    is_ood = scores > thr
    return scores, is_ood, topk_ids


def kernel(z_hyp, leaf_emb, leaf_node_ids, threshold):
    in_maps, (z, y, x2, y2) = preprocess(z_hyp, leaf_emb)
    results = run_device(in_maps).results
    return postprocess(results, z, y, x2, y2, leaf_node_ids, threshold)


# revision 9
# speedup vs baseline: 1.4392x; 1.4392x over previous
"""Trainium2 Bass kernel for the OOD-detection kNN head (Poincare-ball 5-NN).

Math: for queries z (4096,256) and leaves y (50000,256) inside the unit ball,
the reference ranks leaves per query by the Poincare distance
  d = arccosh(1 + 2*|x-y|^2 / ((1-|x|^2)(1-|y|^2)))
Using the hyperboloid identity,
  cosh d = [(1+x2)(1+y2) - 4 z.y] / [(1-x2)(1-y2)]
per query the ranking is equivalent to maximizing
  H[b,l] = z'_b . Y4_l - Y0[l]
with z' = z/(1+x2) (host precompute), Y4 = 4*y/(1-y2), Y0 = (1+y2)/(1-y2).
The device computes H with fp16 matmuls (K=256 dot + a K=2 matmul adding the
hi/lo-split fp16 bias -Y0), reduces H by max over groups of G=32 consecutive
leaves (DVE tensor_reduce straight out of PSUM), and returns the top-8 group
values + indices per query per core (nc.vector.max / max_index). The host
merges the 64 candidate groups per query, recomputes exact fp32 distances for
members of the best groups, and emits top-5 ids / min score / OOD flag.
fp16 ranking noise (~6e-4) is far below the group-selection margins (the
true top-5 groups rank <=5 of the 8 kept, validated on the fixed dataset).

Sharding: leaves split across 8 cores (padded 50000 -> 8*6656), queries
replicated. Output per core is tiny (4096 x 8 x 2), merged on host.
"""

import sys

for _p in ("/opt/trn_rl_repo", "/root/.axon_site/_ro/trn_rl_repo"):
    if _p not in sys.path:
        sys.path.append(_p)

import numpy as np

import concourse.bacc as bacc
import concourse.mybir as mybir
from concourse.tile import TileContext

B = 4096          # queries
D = 256           # embedding dim
L = 50000         # real leaves
NCORES = 8
LSH = 6656        # leaves per core (padded)
LPAD = NCORES * LSH
G = 32            # leaves per candidate group
NGROUP = LSH // G  # 208 groups per core
P = 128
NB = B // P       # 32 query blocks
PSW = 1664        # leaf columns per PSUM tile (4 banks)
NPS = LSH // PSW  # 4 PSUM tiles per query block
GPT = PSW // G    # 52 groups per PSUM tile
CHUNK = 512       # moving columns per matmul
TOPG = 8          # merged groups whose members get exact recompute
EPS = 1e-7
BIAS_C = 6.0      # bias recentering so hi/lo fp16 split is well-scaled
PAD_BIAS = -1000.0

MMDT = mybir.dt.float16
F32 = mybir.dt.float32
U32 = mybir.dt.uint32


def build_nc():
    nc = bacc.Bacc()
    zs_d = nc.declare_dram_parameter("zs", [2, P, B], MMDT, isOutput=False)
    lf_d = nc.declare_dram_parameter("lf", [2, P, LSH], MMDT, isOutput=False)
    bias_d = nc.declare_dram_parameter("bias", [2, LSH], MMDT, isOutput=False)
    ones_d = nc.declare_dram_parameter("ones", [2, P], MMDT, isOutput=False)
    outv_d = nc.declare_dram_parameter("outv", [NB, P, 8], F32, isOutput=True)
    outi_d = nc.declare_dram_parameter("outi", [NB, P, 8], U32, isOutput=True)

    with TileContext(nc) as tc:
        with (
            tc.tile_pool(name="const", bufs=1) as cpool,
            tc.tile_pool(name="zp", bufs=1) as zpool,
            tc.tile_pool(name="lp", bufs=1) as lpool,
            tc.tile_pool(name="red", bufs=3) as rpool,
            tc.tile_pool(name="o8", bufs=4) as opool,
            tc.tile_pool(name="ps", bufs=2, space="PSUM") as ppool,
        ):
            ones_t = cpool.tile([2, P], MMDT)
            nc.sync.dma_start(out=ones_t, in_=ones_d[:, :])
            bias_t = cpool.tile([2, LSH], MMDT)
            nc.sync.dma_start(out=bias_t, in_=bias_d[:, :])

            z_t = zpool.tile([P, 2, B], MMDT)
            nc.sync.dma_start(out=z_t, in_=zs_d.rearrange("h p b -> p h b"))

            leaf_ts = []
            for tt in range(NPS):
                lt = lpool.tile([P, 2, PSW], MMDT, name=f"leaf{tt}")
                nc.sync.dma_start(
                    out=lt,
                    in_=lf_d[:, :, tt * PSW:(tt + 1) * PSW].rearrange(
                        "h p c -> p h c"
                    ),
                )
                leaf_ts.append(lt)

            for bb in range(NB):
                bsl = slice(bb * P, (bb + 1) * P)
                acc = rpool.tile([P, NGROUP], F32, tag="acc")
                for tt in range(NPS):
                    ps = ppool.tile([P, PSW], F32, tag="ps")
                    for c0 in range(0, PSW, CHUNK):
                        w = min(CHUNK, PSW - c0)
                        csl = slice(c0, c0 + w)
                        gsl = slice(tt * PSW + c0, tt * PSW + c0 + w)
                        nc.tensor.matmul(
                            ps[:, csl], z_t[:, 0, bsl], leaf_ts[tt][:, 0, csl],
                            start=True, stop=False,
                        )
                        nc.tensor.matmul(
                            ps[:, csl], z_t[:, 1, bsl], leaf_ts[tt][:, 1, csl],
                            start=False, stop=False,
                        )
                        nc.tensor.matmul(
                            ps[:, csl], ones_t, bias_t[:, gsl],
                            start=False, stop=True,
                        )
                    nc.vector.tensor_reduce(
                        out=acc[:, tt * GPT:(tt + 1) * GPT],
                        in_=ps.rearrange("p (j g) -> p j g", g=G),
                        axis=mybir.AxisListType.X,
                        op=mybir.AluOpType.max,
                    )
                vals_t = opool.tile([P, 8], F32, tag="vals")
                idx_t = opool.tile([P, 8], U32, tag="idx")
                nc.vector.max(out=vals_t, in_=acc)
                nc.vector.max_index(out=idx_t, in_max=vals_t, in_values=acc)
                nc.sync.dma_start(out=outv_d[bb], in_=vals_t)
                nc.sync.dma_start(out=outi_d[bb], in_=idx_t)
    nc.finalize()
    return nc


_NC = None


def _get_nc():
    global _NC
    if _NC is None:
        _NC = build_nc()
    return _NC


def preprocess(z_hyp, leaf_emb):
    z = np.ascontiguousarray(z_hyp, dtype=np.float32)
    y = np.ascontiguousarray(leaf_emb, dtype=np.float32)
    x2 = np.einsum("bd,bd->b", z, z, dtype=np.float32)
    zp = (z / (1.0 + x2)[:, None]).astype(np.float16)
    zs = np.ascontiguousarray(zp.T.reshape(2, P, B))

    y2 = np.einsum("ld,ld->l", y, y, dtype=np.float32)
    Y4 = (4.0 * y / (1.0 - y2[:, None])).astype(np.float16)
    Y4p = np.zeros((LPAD, D), np.float16)
    Y4p[:L] = Y4
    nY0C = np.full((LPAD,), PAD_BIAS + BIAS_C, np.float64)
    nY0C[:L] = BIAS_C - (
        (1.0 + y2.astype(np.float64)) / (1.0 - y2.astype(np.float64))
    )
    bh = nY0C.astype(np.float16)
    bl = (nY0C - bh.astype(np.float64)).astype(np.float16)

    in_maps = []
    for c in range(NCORES):
        sl = slice(c * LSH, (c + 1) * LSH)
        lf = np.ascontiguousarray(Y4p[sl].T.reshape(2, P, LSH))
        bias = np.ascontiguousarray(np.stack([bh[sl], bl[sl]]))
        in_maps.append(
            {
                "zs": zs,
                "lf": lf,
                "bias": bias,
                "ones": np.ones((2, P), np.float16),
            }
        )
    return in_maps, (z, y, x2, y2)


def run_device(in_maps, trace=False, **kw):
    from concourse.bass_utils import run_bass_kernel_spmd

    return run_bass_kernel_spmd(
        _get_nc(), in_maps, list(range(NCORES)), trace=trace, **kw
    )


def postprocess(results, z, y, x2, y2, leaf_node_ids, threshold):
    ids = np.asarray(leaf_node_ids)
    thr = np.float32(threshold)

    vals = np.stack(
        [results[c]["outv"].reshape(B, 8) for c in range(NCORES)], axis=1
    ).reshape(B, NCORES * 8)
    gidx = np.stack(
        [
            results[c]["outi"].reshape(B, 8).astype(np.int64) + c * NGROUP
            for c in range(NCORES)
        ],
        axis=1,
    ).reshape(B, NCORES * 8)

    order = np.argsort(-vals, axis=1, kind="stable")[:, :TOPG]
    sel = np.take_along_axis(gidx, order, axis=1)            # (B, TOPG)

    leaf_idx = (sel[:, :, None] * G + np.arange(G)[None, None, :]).reshape(
        B, TOPG * G
    )
    valid = leaf_idx < L
    li = np.where(valid, leaf_idx, 0)

    scores = np.empty(B, np.float32)
    topk_pos = np.empty((B, 5), np.int64)
    CH = 512
    for b0 in range(0, B, CH):
        b1 = min(b0 + CH, B)
        lic = li[b0:b1]
        yl = y[lic]                                           # (ch, TG, 256)
        xy = np.einsum("bd,bkd->bk", z[b0:b1], yl, dtype=np.float32)
        sq = np.maximum(
            x2[b0:b1][:, None] + y2[lic] - 2.0 * xy, 0.0
        ).astype(np.float32)
        den = np.maximum(
            (1.0 - x2[b0:b1][:, None]) * (1.0 - y2[lic]), EPS
        ).astype(np.float32)
        arg = np.maximum(1.0 + 2.0 * sq / den, 1.0 + EPS).astype(np.float32)
        d = np.arccosh(arg).astype(np.float32)
        d = np.where(valid[b0:b1], d, np.float32(np.inf))
        ord2 = np.lexsort((lic, d), axis=1)[:, :5]
        topk_pos[b0:b1] = np.take_along_axis(lic, ord2, axis=1)
        scores[b0:b1] = np.take_along_axis(d, ord2, axis=1)[:, 0]

    topk_ids = ids[topk_pos].astype(np.int32)
    is_ood = scores > thr
    return scores, is_ood, topk_ids


def kernel(z_hyp, leaf_emb, leaf_node_ids, threshold):
    in_maps, (z, y, x2, y2) = preprocess(z_hyp, leaf_emb)
    results = run_device(in_maps).results
    return postprocess(results, z, y, x2, y2, leaf_node_ids, threshold)


# revision 12
# speedup vs baseline: 3.1390x; 2.1811x over previous
"""Trainium2 Bass kernel for the OOD-detection kNN head (Poincare-ball 5-NN).

Math: for queries z (4096,256) and leaves y (50000,256) inside the unit ball,
the reference ranks leaves per query by the Poincare distance
  d = arccosh(1 + 2*|x-y|^2 / ((1-|x|^2)(1-|y|^2)))
Using the hyperboloid identity,
  cosh d = [(1+x2)(1+y2) - 4 z.y] / [(1-x2)(1-y2)]
per query the ranking is equivalent to maximizing
  H[b,l] = z'_b . Y4_l - Y0[l]
with z' = z/(1+x2) (host precompute), Y4 = 4*y/(1-y2), Y0 = (1+y2)/(1-y2).
The device computes H with fp16 matmuls (K=256 dot + a K=2 matmul adding the
hi/lo-split fp16 bias -Y0), reduces H by max over groups of G=32 consecutive
leaves (DVE tensor_reduce straight out of PSUM), and returns the top-8 group
values + indices per query per core (nc.vector.max / max_index). The host
merges the 64 candidate groups per query, recomputes exact fp32 distances for
members of the best groups, and emits top-5 ids / min score / OOD flag.
fp16 ranking noise (~6e-4) is far below the group-selection margins (the
true top-5 groups rank <=5 of the 8 kept, validated on the fixed dataset).

Sharding: leaves split across 8 cores (padded 50000 -> 8*6656), queries
replicated. Output per core is tiny (4096 x 8 x 2), merged on host.
"""

import sys

for _p in ("/opt/trn_rl_repo", "/root/.axon_site/_ro/trn_rl_repo"):
    if _p not in sys.path:
        sys.path.append(_p)

import numpy as np

import concourse.bacc as bacc
import concourse.mybir as mybir
from concourse.tile import TileContext

B = 4096          # queries
D = 256           # embedding dim
L = 50000         # real leaves
NCORES = 8
LSH = 6656        # leaves per core (padded)
LPAD = NCORES * LSH
G = 32            # leaves per candidate group
NGROUP = LSH // G  # 208 groups per core
P = 128
NB = B // P       # 32 query blocks
PSW = 1664        # leaf columns per PSUM tile (4 banks)
NPS = LSH // PSW  # 4 PSUM tiles per query block
GPT = PSW // G    # 52 groups per PSUM tile
CHUNK = 512       # moving columns per matmul
TOPG = 8          # merged groups whose members get exact recompute
EPS = 1e-7
BIAS_C = 6.0      # bias recentering so hi/lo fp16 split is well-scaled
PAD_BIAS = -1000.0

MMDT = mybir.dt.float16
F32 = mybir.dt.float32
U32 = mybir.dt.uint32


def build_nc():
    nc = bacc.Bacc()
    # bias/ones padded to K=128: a K=2 matmul would flip the PE into 32-row
    # tile config between chunks and break matmul pipelining (3x slowdown).
    zs_d = nc.declare_dram_parameter("zs", [2, P, B], MMDT, isOutput=False)
    lf_d = nc.declare_dram_parameter("lf", [2, P, LSH], MMDT, isOutput=False)
    bias_d = nc.declare_dram_parameter("bias", [P, LSH], MMDT, isOutput=False)
    ones_d = nc.declare_dram_parameter("ones", [P, P], MMDT, isOutput=False)
    outv_d = nc.declare_dram_parameter("outv", [NB, P, 8], F32, isOutput=True)
    outi_d = nc.declare_dram_parameter("outi", [NB, P, 8], U32, isOutput=True)

    with TileContext(nc) as tc:
        with (
            tc.tile_pool(name="const", bufs=1) as cpool,
            tc.tile_pool(name="zp", bufs=1) as zpool,
            tc.tile_pool(name="lp", bufs=1) as lpool,
            tc.tile_pool(name="red", bufs=3) as rpool,
            tc.tile_pool(name="o8", bufs=4) as opool,
            tc.tile_pool(name="ps", bufs=2, space="PSUM") as ppool,
        ):
            ones_t = cpool.tile([P, P], MMDT)
            nc.sync.dma_start(out=ones_t, in_=ones_d[:, :])
            bias_t = cpool.tile([P, LSH], MMDT)
            nc.sync.dma_start(out=bias_t, in_=bias_d[:, :])

            z_t = zpool.tile([P, 2, B], MMDT)
            nc.sync.dma_start(out=z_t, in_=zs_d.rearrange("h p b -> p h b"))

            leaf_ts = []
            for tt in range(NPS):
                lt = lpool.tile([P, 2, PSW], MMDT, name=f"leaf{tt}")
                nc.sync.dma_start(
                    out=lt,
                    in_=lf_d[:, :, tt * PSW:(tt + 1) * PSW].rearrange(
                        "h p c -> p h c"
                    ),
                )
                leaf_ts.append(lt)

            for bb in range(NB):
                bsl = slice(bb * P, (bb + 1) * P)
                acc = rpool.tile([P, NGROUP], F32, tag="acc")
                for tt in range(NPS):
                    ps = ppool.tile([P, PSW], F32, tag="ps")
                    for c0 in range(0, PSW, CHUNK):
                        w = min(CHUNK, PSW - c0)
                        csl = slice(c0, c0 + w)
                        gsl = slice(tt * PSW + c0, tt * PSW + c0 + w)
                        nc.tensor.matmul(
                            ps[:, csl], z_t[:, 0, bsl], leaf_ts[tt][:, 0, csl],
                            start=True, stop=False,
                        )
                        nc.tensor.matmul(
                            ps[:, csl], z_t[:, 1, bsl], leaf_ts[tt][:, 1, csl],
                            start=False, stop=False,
                        )
                        nc.tensor.matmul(
                            ps[:, csl], ones_t, bias_t[:, gsl],
                            start=False, stop=True,
                        )
                    nc.vector.tensor_reduce(
                        out=acc[:, tt * GPT:(tt + 1) * GPT],
                        in_=ps.rearrange("p (j g) -> p j g", g=G),
                        axis=mybir.AxisListType.X,
                        op=mybir.AluOpType.max,
                    )
                vals_t = opool.tile([P, 8], F32, tag="vals")
                idx_t = opool.tile([P, 8], U32, tag="idx")
                nc.vector.max(out=vals_t, in_=acc)
                nc.vector.max_index(out=idx_t, in_max=vals_t, in_values=acc)
                nc.sync.dma_start(out=outv_d[bb], in_=vals_t)
                nc.sync.dma_start(out=outi_d[bb], in_=idx_t)
    nc.finalize()
    return nc


_NC = None


def _get_nc():
    global _NC
    if _NC is None:
        _NC = build_nc()
    return _NC


def preprocess(z_hyp, leaf_emb):
    z = np.ascontiguousarray(z_hyp, dtype=np.float32)
    y = np.ascontiguousarray(leaf_emb, dtype=np.float32)
    x2 = np.einsum("bd,bd->b", z, z, dtype=np.float32)
    zp = (z / (1.0 + x2)[:, None]).astype(np.float16)
    zs = np.ascontiguousarray(zp.T.reshape(2, P, B))

    y2 = np.einsum("ld,ld->l", y, y, dtype=np.float32)
    Y4 = (4.0 * y / (1.0 - y2[:, None])).astype(np.float16)
    Y4p = np.zeros((LPAD, D), np.float16)
    Y4p[:L] = Y4
    nY0C = np.full((LPAD,), PAD_BIAS + BIAS_C, np.float64)
    nY0C[:L] = BIAS_C - (
        (1.0 + y2.astype(np.float64)) / (1.0 - y2.astype(np.float64))
    )
    bh = nY0C.astype(np.float16)
    bl = (nY0C - bh.astype(np.float64)).astype(np.float16)
    ones = np.zeros((P, P), np.float16)
    ones[0:2, :] = 1.0

    in_maps = []
    for c in range(NCORES):
        sl = slice(c * LSH, (c + 1) * LSH)
        lf = np.ascontiguousarray(Y4p[sl].T.reshape(2, P, LSH))
        bias = np.zeros((P, LSH), np.float16)
        bias[0] = bh[sl]
        bias[1] = bl[sl]
        in_maps.append({"zs": zs, "lf": lf, "bias": bias, "ones": ones})
    return in_maps, (z, y, x2, y2)


def run_device(in_maps, trace=False, **kw):
    from concourse.bass_utils import run_bass_kernel_spmd

    return run_bass_kernel_spmd(
        _get_nc(), in_maps, list(range(NCORES)), trace=trace, **kw
    )


def postprocess(results, z, y, x2, y2, leaf_node_ids, threshold):
    ids = np.asarray(leaf_node_ids)
    thr = np.float32(threshold)

    vals = np.stack(
        [results[c]["outv"].reshape(B, 8) for c in range(NCORES)], axis=1
    ).reshape(B, NCORES * 8)
    gidx = np.stack(
        [
            results[c]["outi"].reshape(B, 8).astype(np.int64) + c * NGROUP
            for c in range(NCORES)
        ],
        axis=1,
    ).reshape(B, NCORES * 8)

    order = np.argsort(-vals, axis=1, kind="stable")[:, :TOPG]
    sel = np.take_along_axis(gidx, order, axis=1)            # (B, TOPG)

    leaf_idx = (sel[:, :, None] * G + np.arange(G)[None, None, :]).reshape(
        B, TOPG * G
    )
    valid = leaf_idx < L
    li = np.where(valid, leaf_idx, 0)

    scores = np.empty(B, np.float32)
    topk_pos = np.empty((B, 5), np.int64)
    CH = 512
    for b0 in range(0, B, CH):
        b1 = min(b0 + CH, B)
        lic = li[b0:b1]
        yl = y[lic]                                           # (ch, TG, 256)
        xy = np.einsum("bd,bkd->bk", z[b0:b1], yl, dtype=np.float32)
        sq = np.maximum(
            x2[b0:b1][:, None] + y2[lic] - 2.0 * xy, 0.0
        ).astype(np.float32)
        den = np.maximum(
            (1.0 - x2[b0:b1][:, None]) * (1.0 - y2[lic]), EPS
        ).astype(np.float32)
        arg = np.maximum(1.0 + 2.0 * sq / den, 1.0 + EPS).astype(np.float32)
        d = np.arccosh(arg).astype(np.float32)
        d = np.where(valid[b0:b1], d, np.float32(np.inf))
        ord2 = np.lexsort((lic, d), axis=1)[:, :5]
        topk_pos[b0:b1] = np.take_along_axis(lic, ord2, axis=1)
        scores[b0:b1] = np.take_along_axis(d, ord2, axis=1)[:, 0]

    topk_ids = ids[topk_pos].astype(np.int32)
    is_ood = scores > thr
    return scores, is_ood, topk_ids


def kernel(z_hyp, leaf_emb, leaf_node_ids, threshold):
    in_maps, (z, y, x2, y2) = preprocess(z_hyp, leaf_emb)
    results = run_device(in_maps).results
    return postprocess(results, z, y, x2, y2, leaf_node_ids, threshold)


# revision 18
# speedup vs baseline: 3.3187x; 1.0572x over previous
"""Trainium2 Bass kernel for the OOD-detection kNN head (Poincare-ball 5-NN).

Math: for queries z (4096,256) and leaves y (50000,256) inside the unit ball,
the reference ranks leaves per query by the Poincare distance
  d = arccosh(1 + 2*|x-y|^2 / ((1-|x|^2)(1-|y|^2)))
Using the hyperboloid identity,
  cosh d = [(1+x2)(1+y2) - 4 z.y] / [(1-x2)(1-y2)]
per query the ranking is equivalent to maximizing
  H[b,l] = z'_b . Y4_l - Y0[l]
with z' = z/(1+x2) (host precompute), Y4 = 4*y/(1-y2), Y0 = (1+y2)/(1-y2).

Device pipeline (per core, leaves sharded):
  - leaves are HOST-SORTED by Y0 so that each group of G=32 consecutive
    leaves has near-constant Y0; the -Y0 term is then applied per GROUP
    (optimistic bound: -min Y0 of group) after the group-max, instead of a
    third matmul pass per tile.
  - PE: 2 fp16 matmul passes (K=128 each) accumulate z'.Y4 into PSUM.
  - evict: 3 of 4 PSUM tiles are copied by the Scalar engine to fp16 SBUF
    (DVE then reduces them in 2x mode); 1 of 4 is reduced by DVE straight
    from PSUM (fp32). This balances ACT/DVE/PE at ~200us each.
  - DVE: group-max tensor_reduce -> per-group correction add -> top-8
    values+indices per query (nc.vector.max / max_index).
Host: merges 64 candidate groups per query, keeps the best TOPG=12,
recomputes exact fp32 distances for their member leaves (mapped back
through the sort permutation), and emits top-5 ids / min score / OOD flag.
Ranking noise (fp16 inputs + fp16 evict + group Y0 spread) is far below the
group-selection margins (validated on the fixed dataset: worst needed rank
4 of 8 per core, 6 of 12 merged).

Sharding: leaves split across 8 cores (padded 50000 -> 8*6656), queries
replicated. Output per core is tiny (4096 x 8 x 2), merged on host.
"""

import sys

for _p in ("/opt/trn_rl_repo", "/root/.axon_site/_ro/trn_rl_repo"):
    if _p not in sys.path:
        sys.path.append(_p)

import numpy as np

import concourse.bacc as bacc
import concourse.mybir as mybir
from concourse.tile import TileContext

B = 4096          # queries
D = 256           # embedding dim
L = 50000         # real leaves
NCORES = 8
LSH = 6656        # leaves per core (padded)
LPAD = NCORES * LSH
G = 32            # leaves per candidate group
NGROUP = LSH // G  # 208 groups per core
P = 128
NB = B // P       # 32 query blocks
PSW = 1664        # leaf columns per PSUM tile (4 banks)
NPS = LSH // PSW  # 4 PSUM tiles per query block
GPT = PSW // G    # 52 groups per PSUM tile
CHUNK = 512       # moving columns per matmul
TOPG = 12         # merged groups whose members get exact recompute
EPS = 1e-7
PAD_Y0 = 1000.0

MMDT = mybir.dt.float16
F16 = mybir.dt.float16
F32 = mybir.dt.float32
U32 = mybir.dt.uint32


def build_nc():
    nc = bacc.Bacc()
    zs_d = nc.declare_dram_parameter("zs", [2, P, B], MMDT, isOutput=False)
    lf_d = nc.declare_dram_parameter("lf", [2, P, LSH], MMDT, isOutput=False)
    gb_d = nc.declare_dram_parameter("gb", [P, NGROUP], F16, isOutput=False)
    outv_d = nc.declare_dram_parameter("outv", [NB, P, 8], F16, isOutput=True)
    outi_d = nc.declare_dram_parameter("outi", [NB, P, 8], U32, isOutput=True)

    with TileContext(nc) as tc:
        with (
            tc.tile_pool(name="const", bufs=1) as cpool,
            tc.tile_pool(name="zp", bufs=1) as zpool,
            tc.tile_pool(name="lp", bufs=1) as lpool,
            tc.tile_pool(name="ev", bufs=3) as epool,
            tc.tile_pool(name="red", bufs=3) as rpool,
            tc.tile_pool(name="o8", bufs=4) as opool,
            tc.tile_pool(name="ps", bufs=2, space="PSUM") as ppool,
        ):
            gb_t = cpool.tile([P, NGROUP], F16)
            nc.sync.dma_start(out=gb_t, in_=gb_d[:, :])

            z_t = zpool.tile([P, 2, B], MMDT)
            nc.sync.dma_start(out=z_t, in_=zs_d.rearrange("h p b -> p h b"))

            leaf_ts = []
            for tt in range(NPS):
                lt = lpool.tile([P, 2, PSW], MMDT, name=f"leaf{tt}")
                nc.sync.dma_start(
                    out=lt,
                    in_=lf_d[:, :, tt * PSW:(tt + 1) * PSW].rearrange(
                        "h p c -> p h c"
                    ),
                )
                leaf_ts.append(lt)

            for bb in range(NB):
                bsl = slice(bb * P, (bb + 1) * P)
                acc = rpool.tile([P, NGROUP], F16, tag="acc")
                for tt in range(NPS):
                    ps = ppool.tile([P, PSW], F32, tag="ps")
                    for c0 in range(0, PSW, CHUNK):
                        w = min(CHUNK, PSW - c0)
                        csl = slice(c0, c0 + w)
                        nc.tensor.matmul(
                            ps[:, csl], z_t[:, 0, bsl], leaf_ts[tt][:, 0, csl],
                            start=True, stop=False,
                        )
                        nc.tensor.matmul(
                            ps[:, csl], z_t[:, 1, bsl], leaf_ts[tt][:, 1, csl],
                            start=False, stop=True,
                        )
                    osl = slice(tt * GPT, (tt + 1) * GPT)
                    if tt == 0:
                        # direct fp32 reduce from PSUM on DVE
                        nc.vector.tensor_reduce(
                            out=acc[:, osl],
                            in_=ps.rearrange("p (j g) -> p j g", g=G),
                            axis=mybir.AxisListType.X,
                            op=mybir.AluOpType.max,
                        )
                    else:
                        # Scalar-engine evict to fp16, then 2x fp16 reduce
                        hs = epool.tile([P, PSW], F16, tag="hs")
                        nc.scalar.copy(out=hs, in_=ps)
                        nc.vector.tensor_reduce(
                            out=acc[:, osl],
                            in_=hs.rearrange("p (j g) -> p j g", g=G),
                            axis=mybir.AxisListType.X,
                            op=mybir.AluOpType.max,
                        )
                nc.vector.tensor_add(acc, acc, gb_t)
                vals_t = opool.tile([P, 8], F16, tag="vals")
                idx_t = opool.tile([P, 8], U32, tag="idx")
                nc.vector.max(out=vals_t, in_=acc)
                nc.vector.max_index(out=idx_t, in_max=vals_t, in_values=acc)
                nc.sync.dma_start(out=outv_d[bb], in_=vals_t)
                nc.sync.dma_start(out=outi_d[bb], in_=idx_t)
    nc.finalize()
    return nc


_NC = None


def _get_nc():
    global _NC
    if _NC is None:
        _NC = build_nc()
    return _NC


def preprocess(z_hyp, leaf_emb):
    z = np.ascontiguousarray(z_hyp, dtype=np.float32)
    y = np.ascontiguousarray(leaf_emb, dtype=np.float32)
    x2 = np.einsum("bd,bd->b", z, z, dtype=np.float32)
    zp = (z / (1.0 + x2)[:, None]).astype(np.float16)
    zs = np.ascontiguousarray(zp.T.reshape(2, P, B))

    y2 = np.einsum("ld,ld->l", y, y, dtype=np.float32)
    Y0 = (1.0 + y2.astype(np.float64)) / (1.0 - y2.astype(np.float64))
    perm = np.argsort(Y0, kind="stable")
    Y4s = (4.0 * y[perm] / (1.0 - y2[perm][:, None])).astype(np.float16)

    Y4p = np.zeros((LPAD, D), np.float16)
    Y4p[:L] = Y4s
    Y0p = np.full((LPAD,), PAD_Y0, np.float64)
    Y0p[:L] = Y0[perm]
    # optimistic per-group bias, recentered so device values sit near 0
    # (fp16 ulp 1e-3 instead of 4e-3 at the ~6 offset)
    gb = 6.0 - Y0p.reshape(-1, G).min(axis=1)  # (NCORES*NGROUP,)
    gb16 = gb.astype(np.float16)

    in_maps = []
    for c in range(NCORES):
        sl = slice(c * LSH, (c + 1) * LSH)
        lf = np.ascontiguousarray(Y4p[sl].T.reshape(2, P, LSH))
        gbc = np.ascontiguousarray(
            np.broadcast_to(
                gb16[c * NGROUP:(c + 1) * NGROUP][None, :], (P, NGROUP)
            )
        )
        in_maps.append({"zs": zs, "lf": lf, "gb": gbc})
    return in_maps, (z, y, x2, y2, perm)


def run_device(in_maps, trace=False, **kw):
    from concourse.bass_utils import run_bass_kernel_spmd

    return run_bass_kernel_spmd(
        _get_nc(), in_maps, list(range(NCORES)), trace=trace, **kw
    )


def postprocess(results, z, y, x2, y2, perm, leaf_node_ids, threshold):
    ids = np.asarray(leaf_node_ids)
    thr = np.float32(threshold)

    vals = np.stack(
        [
            results[c]["outv"].reshape(B, 8).astype(np.float32)
            for c in range(NCORES)
        ],
        axis=1,
    ).reshape(B, NCORES * 8)
    gidx = np.stack(
        [
            results[c]["outi"].reshape(B, 8).astype(np.int64) + c * NGROUP
            for c in range(NCORES)
        ],
        axis=1,
    ).reshape(B, NCORES * 8)

    order = np.argsort(-vals, axis=1, kind="stable")[:, :TOPG]
    sel = np.take_along_axis(gidx, order, axis=1)            # (B, TOPG)

    spos = (sel[:, :, None] * G + np.arange(G)[None, None, :]).reshape(
        B, TOPG * G
    )                                                        # sorted positions
    valid = spos < L
    li = np.where(valid, perm[np.minimum(spos, L - 1)], 0)   # original ids

    scores = np.empty(B, np.float32)
    topk_pos = np.empty((B, 5), np.int64)
    CH = 512
    for b0 in range(0, B, CH):
        b1 = min(b0 + CH, B)
        lic = li[b0:b1]
        yl = y[lic]                                           # (ch, TG, 256)
        xy = np.einsum("bd,bkd->bk", z[b0:b1], yl, dtype=np.float32)
        sq = np.maximum(
            x2[b0:b1][:, None] + y2[lic] - 2.0 * xy, 0.0
        ).astype(np.float32)
        den = np.maximum(
            (1.0 - x2[b0:b1][:, None]) * (1.0 - y2[lic]), EPS
        ).astype(np.float32)
        arg = np.maximum(1.0 + 2.0 * sq / den, 1.0 + EPS).astype(np.float32)
        d = np.arccosh(arg).astype(np.float32)
        d = np.where(valid[b0:b1], d, np.float32(np.inf))
        ord2 = np.lexsort((lic, d), axis=1)[:, :5]
        topk_pos[b0:b1] = np.take_along_axis(lic, ord2, axis=1)
        scores[b0:b1] = np.take_along_axis(d, ord2, axis=1)[:, 0]

    topk_ids = ids[topk_pos].astype(np.int32)
    is_ood = scores > thr
    return scores, is_ood, topk_ids


def kernel(z_hyp, leaf_emb, leaf_node_ids, threshold):
    in_maps, (z, y, x2, y2, perm) = preprocess(z_hyp, leaf_emb)
    results = run_device(in_maps).results
    return postprocess(results, z, y, x2, y2, perm, leaf_node_ids, threshold)


# revision 21
# speedup vs baseline: 3.3409x; 1.0067x over previous
"""Trainium2 Bass kernel for the OOD-detection kNN head (Poincare-ball 5-NN).

Math: for queries z (4096,256) and leaves y (50000,256) inside the unit ball,
the reference ranks leaves per query by the Poincare distance
  d = arccosh(1 + 2*|x-y|^2 / ((1-|x|^2)(1-|y|^2)))
Using the hyperboloid identity,
  cosh d = [(1+x2)(1+y2) - 4 z.y] / [(1-x2)(1-y2)]
per query the ranking is equivalent to maximizing
  H[b,l] = z'_b . Y4_l - Y0[l]
with z' = z/(1+x2) (host precompute), Y4 = 4*y/(1-y2), Y0 = (1+y2)/(1-y2).

Device pipeline (per core, leaves sharded):
  - leaves are HOST-SORTED by Y0 so that each group of G=32 consecutive
    leaves has near-constant Y0; the -Y0 term is then applied per GROUP
    (optimistic bound: -min Y0 of group) after the group-max, instead of a
    third matmul pass per tile.
  - PE: 2 fp16 matmul passes (K=128 each) accumulate z'.Y4 into PSUM.
  - evict: 3 of 4 PSUM tiles are copied by the Scalar engine to fp16 SBUF
    (DVE then reduces them in 2x mode); 1 of 4 is reduced by DVE straight
    from PSUM (fp32). This balances ACT/DVE/PE at ~200us each.
  - DVE: group-max tensor_reduce -> per-group correction add -> top-8
    values+indices per query (nc.vector.max / max_index).
Host: merges 64 candidate groups per query, keeps the best TOPG=12,
recomputes exact fp32 distances for their member leaves (mapped back
through the sort permutation), and emits top-5 ids / min score / OOD flag.
Ranking noise (fp16 inputs + fp16 evict + group Y0 spread) is far below the
group-selection margins (validated on the fixed dataset: worst needed rank
4 of 8 per core, 6 of 12 merged).

Sharding: leaves split across 8 cores (padded 50000 -> 8*6656), queries
replicated. Output per core is tiny (4096 x 8 x 2), merged on host.
"""

import sys

for _p in ("/opt/trn_rl_repo", "/root/.axon_site/_ro/trn_rl_repo"):
    if _p not in sys.path:
        sys.path.append(_p)

import numpy as np

import concourse.bacc as bacc
import concourse.mybir as mybir
from concourse.tile import TileContext

B = 4096          # queries
D = 256           # embedding dim
L = 50000         # real leaves
NCORES = 8
LSH = 6656        # leaves per core (padded)
LPAD = NCORES * LSH
G = 32            # leaves per candidate group
NGROUP = LSH // G  # 208 groups per core
P = 128
NB = B // P       # 32 query blocks
PSW = 1664        # leaf columns per PSUM tile (4 banks)
NPS = LSH // PSW  # 4 PSUM tiles per query block
GPT = PSW // G    # 52 groups per PSUM tile
CHUNK = 512       # moving columns per matmul
TOPG = 12         # merged groups whose members get exact recompute
EPS = 1e-7
PAD_Y0 = 1000.0

MMDT = mybir.dt.float16
F16 = mybir.dt.float16
F32 = mybir.dt.float32
U32 = mybir.dt.uint32


def build_nc():
    nc = bacc.Bacc()
    zs_d = nc.declare_dram_parameter("zs", [2, P, B], MMDT, isOutput=False)
    lf_d = nc.declare_dram_parameter("lf", [2, P, LSH], MMDT, isOutput=False)
    gb_d = nc.declare_dram_parameter("gb", [P, NGROUP], F16, isOutput=False)
    outv_d = nc.declare_dram_parameter("outv", [NB, P, 8], F16, isOutput=True)
    outi_d = nc.declare_dram_parameter("outi", [NB, P, 8], U32, isOutput=True)

    with TileContext(nc) as tc:
        with (
            tc.tile_pool(name="const", bufs=1) as cpool,
            tc.tile_pool(name="zp", bufs=1) as zpool,
            tc.tile_pool(name="lp", bufs=1) as lpool,
            tc.tile_pool(name="ev", bufs=3) as epool,
            tc.tile_pool(name="red", bufs=3) as rpool,
            tc.tile_pool(name="o8", bufs=4) as opool,
            tc.tile_pool(name="ps", bufs=2, space="PSUM") as ppool,
        ):
            gb_t = cpool.tile([P, NGROUP], F16)
            nc.sync.dma_start(out=gb_t, in_=gb_d[:, :])

            z_t = zpool.tile([P, 2, B], MMDT)
            nc.sync.dma_start(out=z_t, in_=zs_d.rearrange("h p b -> p h b"))

            leaf_ts = []
            for tt in range(NPS):
                lt = lpool.tile([P, 2, PSW], MMDT, name=f"leaf{tt}")
                nc.sync.dma_start(
                    out=lt,
                    in_=lf_d[:, :, tt * PSW:(tt + 1) * PSW].rearrange(
                        "h p c -> p h c"
                    ),
                )
                leaf_ts.append(lt)

            HALF = LSH // 2
            for bb in range(NB):
                bsl = slice(bb * P, (bb + 1) * P)
                acc = rpool.tile([P, NGROUP], F16, tag="acc")
                hrow = epool.tile([P, LSH], F16, tag="hrow")
                for tt in range(NPS):
                    ps = ppool.tile([P, PSW], F32, tag="ps")
                    for c0 in range(0, PSW, CHUNK):
                        w = min(CHUNK, PSW - c0)
                        csl = slice(c0, c0 + w)
                        nc.tensor.matmul(
                            ps[:, csl], z_t[:, 0, bsl], leaf_ts[tt][:, 0, csl],
                            start=True, stop=False,
                        )
                        nc.tensor.matmul(
                            ps[:, csl], z_t[:, 1, bsl], leaf_ts[tt][:, 1, csl],
                            start=False, stop=True,
                        )
                    # Scalar-engine evict to fp16 (DVE tensor_reduce has no
                    # fast mode; tensor_tensor fp16 runs 2x, so fold instead)
                    nc.scalar.copy(
                        out=hrow[:, tt * PSW:(tt + 1) * PSW], in_=ps
                    )
                # contiguous-half fold tree (fp16 2x): group membership is
                # restored by the host-side column permutation
                f1 = epool.tile([P, HALF], F16, tag="f1")
                nc.vector.tensor_tensor(
                    out=f1, in0=hrow[:, :HALF], in1=hrow[:, HALF:],
                    op=mybir.AluOpType.max,
                )
                f2 = epool.tile([P, HALF // 2], F16, tag="f2")
                nc.vector.tensor_tensor(
                    out=f2, in0=f1[:, :HALF // 2], in1=f1[:, HALF // 2:],
                    op=mybir.AluOpType.max,
                )
                f3 = epool.tile([P, HALF // 4], F16, tag="f3")
                nc.vector.tensor_tensor(
                    out=f3, in0=f2[:, :HALF // 4], in1=f2[:, HALF // 4:],
                    op=mybir.AluOpType.max,
                )
                nc.vector.tensor_reduce(
                    out=acc,
                    in_=f3.rearrange("p (j g) -> p j g", g=4),
                    axis=mybir.AxisListType.X,
                    op=mybir.AluOpType.max,
                )
                nc.vector.tensor_add(acc, acc, gb_t)
                vals_t = opool.tile([P, 8], F16, tag="vals")
                idx_t = opool.tile([P, 8], U32, tag="idx")
                nc.vector.max(out=vals_t, in_=acc)
                nc.vector.max_index(out=idx_t, in_max=vals_t, in_values=acc)
                nc.sync.dma_start(out=outv_d[bb], in_=vals_t)
                nc.sync.dma_start(out=outi_d[bb], in_=idx_t)
    nc.finalize()
    return nc


_NC = None


def _get_nc():
    global _NC
    if _NC is None:
        _NC = build_nc()
    return _NC


def col_of():
    """Device column for each within-core sorted position: places group
    g's 32 members so the contiguous-half fold tree reunites them."""
    s = np.arange(LSH)
    g = s // G
    u = s % G
    return (
        4 * g + (u & 3) + 832 * ((u >> 2) & 1)
        + 1664 * ((u >> 3) & 1) + 3328 * ((u >> 4) & 1)
    )


def preprocess(z_hyp, leaf_emb):
    z = np.ascontiguousarray(z_hyp, dtype=np.float32)
    y = np.ascontiguousarray(leaf_emb, dtype=np.float32)
    x2 = np.einsum("bd,bd->b", z, z, dtype=np.float32)
    zp = (z / (1.0 + x2)[:, None]).astype(np.float16)
    zs = np.ascontiguousarray(zp.T.reshape(2, P, B))

    y2 = np.einsum("ld,ld->l", y, y, dtype=np.float32)
    Y0 = (1.0 + y2.astype(np.float64)) / (1.0 - y2.astype(np.float64))
    perm = np.argsort(Y0, kind="stable")
    Y4s = (4.0 * y[perm] / (1.0 - y2[perm][:, None])).astype(np.float16)

    Y4p = np.zeros((LPAD, D), np.float16)
    Y4p[:L] = Y4s
    Y0p = np.full((LPAD,), PAD_Y0, np.float64)
    Y0p[:L] = Y0[perm]
    # optimistic per-group bias, recentered so device values sit near 0
    # (fp16 ulp 1e-3 instead of 4e-3 at the ~6 offset)
    gb = 6.0 - Y0p.reshape(-1, G).min(axis=1)  # (NCORES*NGROUP,)
    gb16 = gb.astype(np.float16)

    cols = col_of()
    in_maps = []
    for c in range(NCORES):
        sl = slice(c * LSH, (c + 1) * LSH)
        Y4c = np.empty((LSH, D), np.float16)
        Y4c[cols] = Y4p[sl]
        lf = np.ascontiguousarray(Y4c.T.reshape(2, P, LSH))
        gbc = np.ascontiguousarray(
            np.broadcast_to(
                gb16[c * NGROUP:(c + 1) * NGROUP][None, :], (P, NGROUP)
            )
        )
        in_maps.append({"zs": zs, "lf": lf, "gb": gbc})
    return in_maps, (z, y, x2, y2, perm)


def run_device(in_maps, trace=False, **kw):
    from concourse.bass_utils import run_bass_kernel_spmd

    return run_bass_kernel_spmd(
        _get_nc(), in_maps, list(range(NCORES)), trace=trace, **kw
    )


def postprocess(results, z, y, x2, y2, perm, leaf_node_ids, threshold):
    ids = np.asarray(leaf_node_ids)
    thr = np.float32(threshold)

    vals = np.stack(
        [
            results[c]["outv"].reshape(B, 8).astype(np.float32)
            for c in range(NCORES)
        ],
        axis=1,
    ).reshape(B, NCORES * 8)
    gidx = np.stack(
        [
            results[c]["outi"].reshape(B, 8).astype(np.int64) + c * NGROUP
            for c in range(NCORES)
        ],
        axis=1,
    ).reshape(B, NCORES * 8)

    order = np.argsort(-vals, axis=1, kind="stable")[:, :TOPG]
    sel = np.take_along_axis(gidx, order, axis=1)            # (B, TOPG)

    spos = (sel[:, :, None] * G + np.arange(G)[None, None, :]).reshape(
        B, TOPG * G
    )                                                        # sorted positions
    valid = spos < L
    li = np.where(valid, perm[np.minimum(spos, L - 1)], 0)   # original ids

    scores = np.empty(B, np.float32)
    topk_pos = np.empty((B, 5), np.int64)
    CH = 512
    for b0 in range(0, B, CH):
        b1 = min(b0 + CH, B)
        lic = li[b0:b1]
        yl = y[lic]                                           # (ch, TG, 256)
        xy = np.einsum("bd,bkd->bk", z[b0:b1], yl, dtype=np.float32)
        sq = np.maximum(
            x2[b0:b1][:, None] + y2[lic] - 2.0 * xy, 0.0
        ).astype(np.float32)
        den = np.maximum(
            (1.0 - x2[b0:b1][:, None]) * (1.0 - y2[lic]), EPS
        ).astype(np.float32)
        arg = np.maximum(1.0 + 2.0 * sq / den, 1.0 + EPS).astype(np.float32)
        d = np.arccosh(arg).astype(np.float32)
        d = np.where(valid[b0:b1], d, np.float32(np.inf))
        ord2 = np.lexsort((lic, d), axis=1)[:, :5]
        topk_pos[b0:b1] = np.take_along_axis(lic, ord2, axis=1)
        scores[b0:b1] = np.take_along_axis(d, ord2, axis=1)[:, 0]

    topk_ids = ids[topk_pos].astype(np.int32)
    is_ood = scores > thr
    return scores, is_ood, topk_ids


def kernel(z_hyp, leaf_emb, leaf_node_ids, threshold):
    in_maps, (z, y, x2, y2, perm) = preprocess(z_hyp, leaf_emb)
    results = run_device(in_maps).results
    return postprocess(results, z, y, x2, y2, perm, leaf_node_ids, threshold)


# revision 23
# speedup vs baseline: 3.8603x; 1.1555x over previous
"""Trainium2 Bass kernel for the OOD-detection kNN head (Poincare-ball 5-NN).

Math: for queries z (4096,256) and leaves y (50000,256) inside the unit ball,
the reference ranks leaves per query by the Poincare distance
  d = arccosh(1 + 2*|x-y|^2 / ((1-|x|^2)(1-|y|^2)))
Using the hyperboloid identity,
  cosh d = [(1+x2)(1+y2) - 4 z.y] / [(1-x2)(1-y2)]
per query the ranking is equivalent to maximizing
  H[b,l] = z'_b . Y4_l - Y0[l]
with z' = z/(1+x2) (host precompute), Y4 = 4*y/(1-y2), Y0 = (1+y2)/(1-y2).

Device pipeline (per core, leaves sharded):
  - leaves are HOST-SORTED by Y0 so that each group of G=32 consecutive
    leaves has near-constant Y0; the -Y0 term is then applied per GROUP
    (optimistic bound: -min Y0 of group) after the group-max, instead of a
    third matmul pass per tile.
  - PE: 2 fp16 matmul passes (K=128 each) accumulate z'.Y4 into PSUM.
  - evict: 3 of 4 PSUM tiles are copied by the Scalar engine to fp16 SBUF
    (DVE then reduces them in 2x mode); 1 of 4 is reduced by DVE straight
    from PSUM (fp32). This balances ACT/DVE/PE at ~200us each.
  - DVE: group-max tensor_reduce -> per-group correction add -> top-8
    values+indices per query (nc.vector.max / max_index).
Host: merges 64 candidate groups per query, keeps the best TOPG=12,
recomputes exact fp32 distances for their member leaves (mapped back
through the sort permutation), and emits top-5 ids / min score / OOD flag.
Ranking noise (fp16 inputs + fp16 evict + group Y0 spread) is far below the
group-selection margins (validated on the fixed dataset: worst needed rank
4 of 8 per core, 6 of 12 merged).

Sharding: leaves split across 8 cores (padded 50000 -> 8*6656), queries
replicated. Output per core is tiny (4096 x 8 x 2), merged on host.
"""

import sys

for _p in ("/opt/trn_rl_repo", "/root/.axon_site/_ro/trn_rl_repo"):
    if _p not in sys.path:
        sys.path.append(_p)

import numpy as np

import concourse.bacc as bacc
import concourse.mybir as mybir
from concourse.tile import TileContext

B = 4096          # queries
D = 256           # embedding dim
L = 50000         # real leaves
NCORES = 8
LSH = 6656        # leaves per core (padded)
LPAD = NCORES * LSH
G = 32            # leaves per candidate group
NGROUP = LSH // G  # 208 groups per core
P = 128
NB = B // P       # 32 query blocks
PSW = 1664        # leaf columns per PSUM tile (4 banks)
NPS = LSH // PSW  # 4 PSUM tiles per query block
GPT = PSW // G    # 52 groups per PSUM tile
CHUNK = 512       # moving columns per matmul
TOPG = 12         # merged groups whose members get exact recompute
EPS = 1e-7
PAD_Y0 = 1000.0

MMDT = mybir.dt.float16
F16 = mybir.dt.float16
F32 = mybir.dt.float32
U32 = mybir.dt.uint32


def build_nc():
    nc = bacc.Bacc()
    zs_d = nc.declare_dram_parameter("zs", [2, P, B], MMDT, isOutput=False)
    lf_d = nc.declare_dram_parameter("lf", [2, P, LSH], MMDT, isOutput=False)
    gb_d = nc.declare_dram_parameter("gb", [P, NGROUP], F16, isOutput=False)
    outv_d = nc.declare_dram_parameter("outv", [NB, P, 8], F16, isOutput=True)
    outi_d = nc.declare_dram_parameter("outi", [NB, P, 8], U32, isOutput=True)

    with TileContext(nc) as tc:
        with (
            tc.tile_pool(name="const", bufs=1) as cpool,
            tc.tile_pool(name="zp", bufs=1) as zpool,
            tc.tile_pool(name="lp", bufs=1) as lpool,
            tc.tile_pool(name="ev", bufs=3) as epool,
            tc.tile_pool(name="red", bufs=3) as rpool,
            tc.tile_pool(name="o8", bufs=4) as opool,
            tc.tile_pool(name="ps", bufs=2, space="PSUM") as ppool,
        ):
            gb_t = cpool.tile([P, NGROUP], F16)
            nc.sync.dma_start(out=gb_t, in_=gb_d[:, :])

            z_t = zpool.tile([P, 2, B], MMDT)
            nc.sync.dma_start(out=z_t, in_=zs_d.rearrange("h p b -> p h b"))

            leaf_ts = []
            for tt in range(NPS):
                lt = lpool.tile([P, 2, PSW], MMDT, name=f"leaf{tt}")
                nc.sync.dma_start(
                    out=lt,
                    in_=lf_d[:, :, tt * PSW:(tt + 1) * PSW].rearrange(
                        "h p c -> p h c"
                    ),
                )
                leaf_ts.append(lt)

            EV = LSH - PSW       # 4992 columns evicted via Scalar engine
            for bb in range(NB):
                bsl = slice(bb * P, (bb + 1) * P)
                acc = rpool.tile([P, NGROUP], F16, tag="acc")
                hrow = epool.tile([P, EV], F16, tag="hrow")
                for tt in range(NPS):
                    ps = ppool.tile([P, PSW], F32, tag="ps")
                    for c0 in range(0, PSW, CHUNK):
                        w = min(CHUNK, PSW - c0)
                        csl = slice(c0, c0 + w)
                        nc.tensor.matmul(
                            ps[:, csl], z_t[:, 0, bsl], leaf_ts[tt][:, 0, csl],
                            start=True, stop=False,
                        )
                        nc.tensor.matmul(
                            ps[:, csl], z_t[:, 1, bsl], leaf_ts[tt][:, 1, csl],
                            start=False, stop=True,
                        )
                    if tt == 0:
                        # direct fp32 group-max from PSUM on DVE
                        nc.vector.tensor_reduce(
                            out=acc[:, :GPT],
                            in_=ps.rearrange("p (j g) -> p j g", g=G),
                            axis=mybir.AxisListType.X,
                            op=mybir.AluOpType.max,
                        )
                    else:
                        # Scalar-engine evict to fp16 (DVE tensor_reduce has
                        # no fast mode; fp16 tensor_tensor runs 2x, so the
                        # rest of the group-max is a fold tree on DVE)
                        nc.scalar.copy(
                            out=hrow[:, (tt - 1) * PSW:tt * PSW], in_=ps
                        )
                # contiguous-half fold tree (fp16 2x): group membership is
                # restored by the host-side column permutation
                f1 = epool.tile([P, EV // 2], F16, tag="f1")
                nc.vector.tensor_tensor(
                    out=f1, in0=hrow[:, :EV // 2], in1=hrow[:, EV // 2:],
                    op=mybir.AluOpType.max,
                )
                f2 = epool.tile([P, EV // 4], F16, tag="f2")
                nc.vector.tensor_tensor(
                    out=f2, in0=f1[:, :EV // 4], in1=f1[:, EV // 4:],
                    op=mybir.AluOpType.max,
                )
                f3 = epool.tile([P, EV // 8], F16, tag="f3")
                nc.vector.tensor_tensor(
                    out=f3, in0=f2[:, :EV // 8], in1=f2[:, EV // 8:],
                    op=mybir.AluOpType.max,
                )
                nc.vector.tensor_reduce(
                    out=acc[:, GPT:],
                    in_=f3.rearrange("p (j g) -> p j g", g=4),
                    axis=mybir.AxisListType.X,
                    op=mybir.AluOpType.max,
                )
                nc.vector.tensor_add(acc, acc, gb_t)
                vals_t = opool.tile([P, 8], F16, tag="vals")
                idx_t = opool.tile([P, 8], U32, tag="idx")
                nc.vector.max(out=vals_t, in_=acc)
                nc.vector.max_index(out=idx_t, in_max=vals_t, in_values=acc)
                nc.sync.dma_start(out=outv_d[bb], in_=vals_t)
                nc.sync.dma_start(out=outi_d[bb], in_=idx_t)
    nc.finalize()
    return nc


_NC = None


def _get_nc():
    global _NC
    if _NC is None:
        _NC = build_nc()
    return _NC


def col_of():
    """Device column for each within-core sorted position.

    Groups 0..51 live in PSUM tile 0 as consecutive 32-column runs (direct
    DVE reduce); groups 52..207 are scattered over columns 1664..6655 so the
    contiguous-half fold tree over the evicted 4992 columns reunites them."""
    s = np.arange(LSH)
    g = s // G
    u = s % G
    j = g - GPT
    folded = (
        PSW + 4 * j + (u & 3) + 624 * ((u >> 2) & 1)
        + 1248 * ((u >> 3) & 1) + 2496 * ((u >> 4) & 1)
    )
    return np.where(g < GPT, G * g + u, folded)


def preprocess(z_hyp, leaf_emb):
    z = np.ascontiguousarray(z_hyp, dtype=np.float32)
    y = np.ascontiguousarray(leaf_emb, dtype=np.float32)
    x2 = np.einsum("bd,bd->b", z, z, dtype=np.float32)
    zp = (z / (1.0 + x2)[:, None]).astype(np.float16)
    zs = np.ascontiguousarray(zp.T.reshape(2, P, B))

    y2 = np.einsum("ld,ld->l", y, y, dtype=np.float32)
    Y0 = (1.0 + y2.astype(np.float64)) / (1.0 - y2.astype(np.float64))
    perm = np.argsort(Y0, kind="stable")
    Y4s = (4.0 * y[perm] / (1.0 - y2[perm][:, None])).astype(np.float16)

    Y4p = np.zeros((LPAD, D), np.float16)
    Y4p[:L] = Y4s
    Y0p = np.full((LPAD,), PAD_Y0, np.float64)
    Y0p[:L] = Y0[perm]
    # optimistic per-group bias, recentered so device values sit near 0
    # (fp16 ulp 1e-3 instead of 4e-3 at the ~6 offset)
    gb = 6.0 - Y0p.reshape(-1, G).min(axis=1)  # (NCORES*NGROUP,)
    gb16 = gb.astype(np.float16)

    cols = col_of()
    in_maps = []
    for c in range(NCORES):
        sl = slice(c * LSH, (c + 1) * LSH)
        Y4c = np.empty((LSH, D), np.float16)
        Y4c[cols] = Y4p[sl]
        lf = np.ascontiguousarray(Y4c.T.reshape(2, P, LSH))
        gbc = np.ascontiguousarray(
            np.broadcast_to(
                gb16[c * NGROUP:(c + 1) * NGROUP][None, :], (P, NGROUP)
            )
        )
        in_maps.append({"zs": zs, "lf": lf, "gb": gbc})
    return in_maps, (z, y, x2, y2, perm)


def run_device(in_maps, trace=False, **kw):
    from concourse.bass_utils import run_bass_kernel_spmd

    return run_bass_kernel_spmd(
        _get_nc(), in_maps, list(range(NCORES)), trace=trace, **kw
    )


def postprocess(results, z, y, x2, y2, perm, leaf_node_ids, threshold):
    ids = np.asarray(leaf_node_ids)
    thr = np.float32(threshold)

    vals = np.stack(
        [
            results[c]["outv"].reshape(B, 8).astype(np.float32)
            for c in range(NCORES)
        ],
        axis=1,
    ).reshape(B, NCORES * 8)
    gidx = np.stack(
        [
            results[c]["outi"].reshape(B, 8).astype(np.int64) + c * NGROUP
            for c in range(NCORES)
        ],
        axis=1,
    ).reshape(B, NCORES * 8)

    order = np.argsort(-vals, axis=1, kind="stable")[:, :TOPG]
    sel = np.take_along_axis(gidx, order, axis=1)            # (B, TOPG)

    spos = (sel[:, :, None] * G + np.arange(G)[None, None, :]).reshape(
        B, TOPG * G
    )                                                        # sorted positions
    valid = spos < L
    li = np.where(valid, perm[np.minimum(spos, L - 1)], 0)   # original ids

    scores = np.empty(B, np.float32)
    topk_pos = np.empty((B, 5), np.int64)
    CH = 512
    for b0 in range(0, B, CH):
        b1 = min(b0 + CH, B)
        lic = li[b0:b1]
        yl = y[lic]                                           # (ch, TG, 256)
        xy = np.einsum("bd,bkd->bk", z[b0:b1], yl, dtype=np.float32)
        sq = np.maximum(
            x2[b0:b1][:, None] + y2[lic] - 2.0 * xy, 0.0
        ).astype(np.float32)
        den = np.maximum(
            (1.0 - x2[b0:b1][:, None]) * (1.0 - y2[lic]), EPS
        ).astype(np.float32)
        arg = np.maximum(1.0 + 2.0 * sq / den, 1.0 + EPS).astype(np.float32)
        d = np.arccosh(arg).astype(np.float32)
        d = np.where(valid[b0:b1], d, np.float32(np.inf))
        ord2 = np.lexsort((lic, d), axis=1)[:, :5]
        topk_pos[b0:b1] = np.take_along_axis(lic, ord2, axis=1)
        scores[b0:b1] = np.take_along_axis(d, ord2, axis=1)[:, 0]

    topk_ids = ids[topk_pos].astype(np.int32)
    is_ood = scores > thr
    return scores, is_ood, topk_ids


def kernel(z_hyp, leaf_emb, leaf_node_ids, threshold):
    in_maps, (z, y, x2, y2, perm) = preprocess(z_hyp, leaf_emb)
    results = run_device(in_maps).results
    return postprocess(results, z, y, x2, y2, perm, leaf_node_ids, threshold)
